# revision 30
# baseline (speedup 1.0000x reference)
"""ConfidenceBiasedCrossAttention Trainium2 kernel (8 NeuronCores), v5.

Sharding (Megatron-style): data-parallel over B (2) x head-parallel over
4 head-groups of 4 heads (256 channels) -> 8 cores. Each core computes
q/k/v projections for its 256 channels, biased softmax attention for its
4 heads, and a partial output projection (rows of Wo). Host sums the 4
partials per batch and adds the bias.

v2: host-side transposes + bf16 casts (no device transposes, half DMA).
v3/v4: multi-queue DMA, interleaved softmax-finish, pipelined tail.
v5: phase-balance PE vs ACT. The exp stream (128 x [128,1024] tiles,
~1.04us each on ACT) exceeds PE matmul work during the attention tail,
so heads are rescheduled:
  - Phase 1 (PE-bound): projections + heads 0 AND 1 fully streamed (two
    PSUM accumulators) + head 2's logits+exp computed and STORED in SBUF
    (e2T, 64KB/part). 96 of 128 exps hide under ~129us of PE work.
  - Phase 2 (PE-bound): head 3 logits/exp live, head 2's AVs replayed
    from e2T (no ACT cost), head 3 AVs lagged. ACT has only 33us left.
  - K/V/Q projection PSUM groups borrow the pl tag, so PSUM is exactly
    8 banks: pl [128,1024]x2 + acc [128,1024]x2.

The V-projection bias is folded into the host-side bias: softmax weights
sum to 1, so attn @ (v + bv) = attn @ v + bv, and sum_g bv_g @ Wo_g.T =
Wv_b @ Wo_w.T is added on the host (exact).
"""

import numpy as np
import ml_dtypes

import concourse.bacc as bacc
import concourse.mybir as mybir
import concourse.tile as tile
from concourse import bass_utils

F32 = mybir.dt.float32
F32R = mybir.dt.float32r
BF16 = mybir.dt.bfloat16
NPBF16 = ml_dtypes.bfloat16
AF = mybir.ActivationFunctionType
MUL = mybir.AluOpType.mult

P = 128
C = 1024
D = 64
LQ = 1024
LK = 4096
CS = 256          # channels per core (4 heads)
NH = 4            # heads per core
SCALE = 1.0 / 8.0
NCH = LK // P     # 32 key chunks of 128
NQT = 4           # key quarters streamed from HBM
QK = LK // NQT    # 1024 keys per quarter


def build_nc():
    nc = bacc.Bacc("TRN2", target_bir_lowering=False, debug=False, num_devices=8)
    qt_d = nc.dram_tensor("qt", [C, LQ], BF16, kind="ExternalInput").ap()
    kt_d = nc.dram_tensor("kt", [C, LK], BF16, kind="ExternalInput").ap()
    vt_d = nc.dram_tensor("vt", [C, LK], BF16, kind="ExternalInput").ap()
    wq_d = nc.dram_tensor("wq", [C, CS], BF16, kind="ExternalInput").ap()
    wk_d = nc.dram_tensor("wk", [C, CS], BF16, kind="ExternalInput").ap()
    wv_d = nc.dram_tensor("wv", [C, CS], BF16, kind="ExternalInput").ap()
    wo_d = nc.dram_tensor("wo", [CS, C], BF16, kind="ExternalInput").ap()
    vbias_d = nc.dram_tensor("vbias", [P, NCH], F32, kind="ExternalInput").ap()
    bq_d = nc.dram_tensor("bq", [P, 2], F32, kind="ExternalInput").ap()
    bk_d = nc.dram_tensor("bk", [P, 2], F32, kind="ExternalInput").ap()
    out_d = nc.dram_tensor("out", [LQ, C], BF16, kind="ExternalOutput").ap()

    with tile.TileContext(nc) as tc:
        with (
            tc.tile_pool(name="pers", bufs=1) as pers,
            tc.tile_pool(name="sb", bufs=1) as sb,
            tc.tile_pool(name="ps", bufs=2, space="PSUM") as ps,
        ):
            # ---- weights (host-transposed: [c_in, c_out]); SP queue ----
            wqs = pers.tile([P, 8, CS], BF16)
            wks = pers.tile([P, 8, CS], BF16)
            wvs = pers.tile([P, 8, CS], BF16)
            wos = pers.tile([P, 2, C], BF16)
            nc.sync.dma_start(wqs, wq_d.rearrange("(t p) o -> p t o", p=P))
            bq_sb = pers.tile([P, 2], F32)
            nc.sync.dma_start(bq_sb, bq_d)
            nc.sync.dma_start(wks, wk_d.rearrange("(t p) o -> p t o", p=P))
            bk_sb = pers.tile([P, 2], F32)
            nc.sync.dma_start(bk_sb, bk_d)
            vbias_sb = pers.tile([P, NCH], F32)
            nc.sync.dma_start(vbias_sb, vbias_d)
            nc.sync.dma_start(wvs, wv_d.rearrange("(t p) o -> p t o", p=P))
            nc.sync.dma_start(wos, wo_d.rearrange("(t p) o -> p t o", p=P))

            # ---- Q staging in quarter-chunks; ACT queue ----
            qtc = []
            for n in range(4):
                t = sb.tile([P, 8, 256], BF16, tag="qstage", bufs=4, name="qtc")
                nc.scalar.dma_start(
                    t, qt_d[:, n * 256 : (n + 1) * 256].rearrange("(t p) r -> p t r", p=P)
                )
                qtc.append(t)

            # ---- constants ----
            ones_f32 = pers.tile([P, 1], F32)
            nc.gpsimd.memset(ones_f32, 1.0)
            ones_bf = pers.tile([P, 1], BF16)
            nc.vector.tensor_copy(ones_bf, ones_f32)
            ones_r = pers.tile([1, P], F32R)
            nc.vector.tensor_copy(ones_r, ones_f32[0:1, :].to_broadcast([1, P]))

            # ---- persistent activations ----
            qT = pers.tile([P, 2, LQ], BF16)        # [ch%128, ch//128, q]
            kT = pers.tile([P, 2, LK], BF16)        # [ch%128, ch//128, key]
            v65 = pers.tile([P, NCH, NH, D + 1], BF16)  # [key%128, chunk, h, v|1]
            attnT = pers.tile([P, 2, LQ], BF16)
            e2T = pers.tile([P, NCH, LQ], BF16)     # head-2 exp store (64KB/part)

            # ones column of v65 (denominator trick)
            nc.vector.tensor_copy(
                v65[:, :, :, D].rearrange("p a b -> p (a b)"),
                ones_bf.to_broadcast([P, NCH * NH]),
            )

            # ---- Q projection (PSUM via pl tag: 4 x [128,512] tiles) ----
            for n in range(4):
                for mt in range(2):
                    pq = ps.tile([P, 512], F32, tag="pl", bufs=4, name="pq")
                    for i in range(8):
                        nc.tensor.matmul(
                            pq[:, 0:256], wqs[:, i, mt * P : (mt + 1) * P],
                            qtc[n][:, i, :],
                            start=(i == 0), stop=(i == 7),
                        )
                    nc.vector.tensor_scalar_add(
                        qT[:, mt, n * 256 : (n + 1) * 256], pq[:, 0:256],
                        bq_sb[:, mt : mt + 1],
                    )

            # ---- K & V projection pieces for one 512-key half; Pool DMA ----
            def half_loads(lo):
                ktq = sb.tile([P, 8, 512], BF16, tag="kstage", bufs=2, name="ktq")
                nc.gpsimd.dma_start(
                    ktq, kt_d[:, lo : lo + 512].rearrange("(t p) k -> p t k", p=P)
                )
                vtq = sb.tile([P, 8, 512], BF16, tag="vstage", bufs=2, name="vtq")
                nc.gpsimd.dma_start(
                    vtq, vt_d[:, lo : lo + 512].rearrange("(t p) k -> p t k", p=P)
                )
                return ktq, vtq

            def kproj(ktq, lo, mt):
                pk = ps.tile([P, 512], F32, tag="pl", bufs=4, name="pk")
                for i in range(8):
                    nc.tensor.matmul(
                        pk, wks[:, i, mt * P : (mt + 1) * P], ktq[:, i, :],
                        start=(i == 0), stop=(i == 7),
                    )
                nc.vector.tensor_scalar_add(
                    kT[:, mt, lo : lo + 512], pk, bk_sb[:, mt : mt + 1]
                )

            def vproj(vtq, c, a):
                pv = ps.tile([P, 512], F32, tag="pl", bufs=4, name="pv")
                for i in range(8):
                    nc.tensor.matmul(
                        pv[:, 0:CS], vtq[:, i, a * P : (a + 1) * P], wvs[:, i, :],
                        start=(i == 0), stop=(i == 7),
                    )
                nc.vector.tensor_copy(
                    v65[:, c, :, 0:D], pv[:, 0:CS].rearrange("p (h d) -> p h d", d=D)
                )

            # ---- attention pieces ----
            e_tiles = {}

            def logits_exp(h, c):
                ht, hp = h // 2, (h % 2) * D
                if h == 2:
                    dsts = (e2T[:, c, 0:512], e2T[:, c, 512:1024])
                else:
                    eT = sb.tile([P, LQ], BF16, tag="exp", bufs=5, name="eT")
                    dsts = (eT[:, 0:512], eT[:, 512:1024])
                    e_tiles[(h, c)] = eT
                for n in range(2):
                    pl = ps.tile([P, 512], F32, tag="pl", bufs=4, name="pl")
                    nc.tensor.matmul(
                        pl,
                        kT[hp : hp + D, ht, c * P : (c + 1) * P],
                        qT[hp : hp + D, ht, n * 512 : (n + 1) * 512],
                        start=True, stop=True, tile_position=(hp, 0),
                    )
                    nc.scalar.activation(
                        dsts[n], pl, AF.Exp, bias=vbias_sb[:, c : c + 1], scale=SCALE
                    )

            def av(h, c, po):
                src = e2T[:, c, :] if h == 2 else e_tiles.pop((h, c))
                for n in range(2):
                    nc.tensor.matmul(
                        po[0 : D + 1, n * 512 : (n + 1) * 512],
                        v65[:, c, h, :],
                        src[:, n * 512 : (n + 1) * 512],
                        start=(c == 0), stop=(c == NCH - 1),
                    )

            def make_finish(h, po):
                """Finish pieces for head h: rec (DVE), pb+bc (PE+DVE),
                tt (DVE). Interleave into the following instruction stream."""
                ht, hp = h // 2, (h % 2) * D
                state = {}

                def rec():
                    r = sb.tile([1, LQ], F32R, tag="rec", bufs=2, name="rec")
                    with nc.allow_low_precision(reason="softmax denom reciprocal"):
                        nc.vector.reciprocal(r, po[D : D + 1, :])
                    state["rec"] = r

                def pb_bc():
                    bc = sb.tile([D, LQ], F32, tag="bc", bufs=2, name="bc")
                    for n in range(2):
                        pb = ps.tile([P, 512], F32, tag="pl", bufs=4, name="pb")
                        nc.tensor.matmul(
                            pb[0:D, :], ones_r[:, 0:D],
                            state["rec"][:, n * 512 : (n + 1) * 512],
                            start=True, stop=True,
                        )
                        nc.vector.tensor_copy(bc[:, n * 512 : (n + 1) * 512], pb[0:D, :])
                    state["bc"] = bc

                def tt():
                    nc.vector.tensor_tensor(
                        attnT[hp : hp + D, ht, :], po[0:D, :], state["bc"], MUL
                    )

                return (rec, pb_bc, tt)

            # ---- phase 1: projections + heads 0,1 streamed + head 2 exps ----
            # Projection PSUM groups (pl-independent PE work for their 8-matmul
            # bodies) are spread between attention chunks so the PE never
            # outruns the 2-buffer pl rotation waiting on ACT; head-2 logits
            # lag one half-block behind to stay spread out.
            po0 = ps.tile([P, LQ], F32, tag="acc", bufs=2, name="po0")
            po1 = ps.tile([P, LQ], F32, tag="acc", bufs=2, name="po1")
            l2q = []
            loads = half_loads(0)
            for hb in range(2 * NQT):
                lo = hb * 512
                ktq, vtq = loads
                kproj(ktq, lo, 0)
                vproj(vtq, hb * 4 + 0, 0)
                for cc in range(4):
                    c = hb * 4 + cc
                    logits_exp(0, c)
                    if c > 0:
                        av(0, c - 1, po0)
                    logits_exp(1, c)
                    if c > 0:
                        av(1, c - 1, po1)
                    if cc == 1 and hb < 2 * NQT - 1:
                        loads = half_loads(lo + 512)
                    if cc < 3:
                        vproj(vtq, c + 1, cc + 1)
                    else:
                        kproj(ktq, lo, 1)
                    l2q.append(c)
                    while len(l2q) > 4:
                        logits_exp(2, l2q.pop(0))
            av(0, NCH - 1, po0)
            av(1, NCH - 1, po1)

            # ---- phase 2: head 3 live + head 2 AV replay; finishes ----
            fin0 = make_finish(0, po0)
            fin1 = make_finish(1, po1)
            po2 = ps.tile([P, LQ], F32, tag="acc", bufs=2, name="po2")
            po3 = ps.tile([P, LQ], F32, tag="acc", bufs=2, name="po3")
            AV2LAG, AV3LAG = 3, 4
            for c in range(NCH):
                logits_exp(3, c)
                if c == 0:
                    fin0[0]()
                elif c == 1:
                    fin0[1]()
                    fin1[0]()
                elif c == 2:
                    fin0[2]()
                    fin1[1]()
                elif c == 3:
                    fin1[2]()
                if l2q:
                    logits_exp(2, l2q.pop(0))
                if c >= AV2LAG:
                    av(2, c - AV2LAG, po2)
                if c >= AV3LAG:
                    av(3, c - AV3LAG, po3)
            for c in range(NCH - AV2LAG, NCH):
                av(2, c, po2)
            for c in range(NCH - AV3LAG, NCH):
                av(3, c, po3)
            fin2 = make_finish(2, po2)
            fin3 = make_finish(3, po3)

            # ---- output projection tail, finishes interleaved ----
            def outproj_halves(m, use_acc):
                if use_acc:
                    big = ps.tile([P, C], F32, tag="acc", bufs=2, name="pw")
                    return (big[:, 0:512], big[:, 512:1024])
                return (
                    ps.tile([P, 512], F32, tag="pl", bufs=4, name="pw"),
                    ps.tile([P, 512], F32, tag="pl", bufs=4, name="pw"),
                )

            def outproj_mm(pws, m, kc):
                for n in range(2):
                    nc.tensor.matmul(
                        pws[n],
                        attnT[:, kc, m * P : (m + 1) * P],
                        wos[:, kc, n * 512 : (n + 1) * 512],
                        start=(kc == 0), stop=(kc == 1),
                    )

            def outproj_store(pws, m):
                ob = sb.tile([P, C], BF16, tag="ob", bufs=3, name="ob")
                nc.vector.tensor_copy(ob[:, 0:512], pws[0])
                nc.scalar.activation(ob[:, 512:1024], pws[1], AF.Copy)
                nc.sync.dma_start(out_d[m * P : (m + 1) * P, 0:512], ob[:, 0:512])
                nc.gpsimd.dma_start(
                    out_d[m * P : (m + 1) * P, 512:1024], ob[:, 512:1024]
                )

            # fin2/fin3 chains overlap each other; m=0's kc=0 accumulation
            # (needs only heads 0-1) overlaps fin3. Later m-tiles alternate
            # pl/acc pools (po2/po3 banks are free after tt).
            fin2[0]()
            fin2[1]()
            fin2[2]()
            pws0 = outproj_halves(0, use_acc=False)
            outproj_mm(pws0, 0, 0)
            fin3[0]()
            pws1 = outproj_halves(1, use_acc=True)  # po2's banks, free after tt2
            outproj_mm(pws1, 1, 0)
            fin3[1]()
            fin3[2]()
            outproj_mm(pws0, 0, 1)
            outproj_store(pws0, 0)
            outproj_mm(pws1, 1, 1)
            outproj_store(pws1, 1)
            for m in range(2, 8):
                pws = outproj_halves(m, use_acc=(m % 2 == 1))
                outproj_mm(pws, m, 0)
                outproj_mm(pws, m, 1)
                outproj_store(pws, m)

    nc.compile()
    return nc


_NC = None


def _get_nc():
    global _NC
    if _NC is None:
        _NC = build_nc()
    return _NC


def shard_inputs(Q, K_in, V_in, V_bias, Wq_w, Wq_b, Wk_w, Wk_b, Wv_w, Wv_b, Wo_w, Wo_b):
    """Build the 8 per-core input dicts (host transposes + bf16 casts)."""
    Q = np.asarray(Q)
    K_in = np.asarray(K_in)
    V_in = np.asarray(V_in)
    V_bias = np.asarray(V_bias)
    per_batch = []
    for b in range(2):
        per_batch.append({
            "qt": np.ascontiguousarray(Q[b].T).astype(NPBF16),
            "kt": np.ascontiguousarray(K_in[b].T).astype(NPBF16),
            "vt": np.ascontiguousarray(V_in[b].T).astype(NPBF16),
            "vbias": np.ascontiguousarray(V_bias[b].reshape(NCH, P).T),
        })
    in_maps = []
    for core in range(8):
        b, g = core // 4, core % 4
        gs, ge = g * CS, (g + 1) * CS
        in_maps.append({
            **per_batch[b],
            "wq": np.ascontiguousarray(Wq_w[gs:ge].T).astype(NPBF16),
            "wk": np.ascontiguousarray(Wk_w[gs:ge].T).astype(NPBF16),
            "wv": np.ascontiguousarray(Wv_w[gs:ge].T).astype(NPBF16),
            "wo": np.ascontiguousarray(Wo_w[:, gs:ge].T).astype(NPBF16),
            "bq": np.ascontiguousarray(Wq_b[gs:ge].reshape(2, P).T),
            "bk": np.ascontiguousarray(Wk_b[gs:ge].reshape(2, P).T),
        })
    return in_maps


def combine_outputs(results, Wv_b, Wo_w, Wo_b):
    """Sum the 4 head-group partials per batch; add output bias and the
    folded V-projection bias (attention weights sum to 1)."""
    bias = Wo_b + Wv_b @ Wo_w.T
    outs = np.stack(
        [np.asarray(r["out"], dtype=np.float32) for r in results]
    ).reshape(2, 4, LQ, C)
    return (outs.sum(axis=1) + bias[None, None, :]).astype(np.float32)


def kernel(**inputs):
    nc = _get_nc()
    in_maps = shard_inputs(**inputs)
    res = bass_utils.run_bass_kernel_spmd(nc, in_maps, core_ids=list(range(8)))
    return combine_outputs(
        res.results,
        np.asarray(inputs["Wv_b"]),
        np.asarray(inputs["Wo_w"]),
        np.asarray(inputs["Wo_b"]),
    )


if __name__ == "__main__":
    rng = np.random.default_rng(0)
    ins = {
        "Q": rng.standard_normal((2, LQ, C), dtype=np.float32),
        "K_in": rng.standard_normal((2, LK, C), dtype=np.float32),
        "V_in": rng.standard_normal((2, LK, C), dtype=np.float32),
        "V_bias": rng.standard_normal((2, LK)).astype(np.float32),
        **{
            f"W{x}_w": (rng.standard_normal((C, C)) * 0.03).astype(np.float32)
            for x in "qkvo"
        },
        **{
            f"W{x}_b": (rng.standard_normal(C) * 0.03).astype(np.float32)
            for x in "qkvo"
        },
    }
    out = kernel(**ins)
    print("ok", out.shape, out.dtype)
